# revision 7
# baseline (speedup 1.0000x reference)
"""GraphSAGE (2x SAGEConv mean-aggr + ReLU) on 8 Trainium2 NeuronCores.

Strategy (graph/data parallel, sharded by destination-node range):
  - Nodes are split into 8 contiguous ranges of N/8; each core owns the
    edges whose dst falls in its range (host sorts edges by dst and packs
    them into 128-node "dst blocks", each padded to M tiles of 128 edges).
  - Messages are gathered per edge with indirect DMA from a replicated
    feature table whose rows are [hi|lo] bf16 splits of the fp32 features
    (512B rows -> full DMA line rate, near-fp32 precision since
    hi + lo == x to ~2^-17 relative).
  - Per edge tile, a 0/1 selection matrix (built on DVE from dst_local vs
    an iota row via is_equal) maps 128 edges -> 128 dst slots; a single
    bf16 matmul per tile accumulates the segment sums in PSUM across the
    block's M tiles.  Sums are scaled by 1/deg, transposed on the PE, and
    pushed through the (fp32) weight matmuls.
  - Layer 1 runs as one SPMD launch producing h (both node-major hi/lo
    rows for layer 2's gather, and feat-major tiles for the local
    x@W_r-style term); the host concatenates the 8 shards and launches
    layer 2 the same way.  Output is produced transposed ([64, n]) and
    transposed back on the host.
"""

import numpy as np
import ml_dtypes

import concourse.bass as bass
import concourse.tile as tile
from concourse import mybir
from concourse.bass_utils import run_bass_kernel_spmd

NCORES = 8
BLK = 128

BF16 = mybir.dt.bfloat16
F32 = mybir.dt.float32
I32 = mybir.dt.int32

# test.py can flip this to capture neuron-profile exec times into LAST
PROFILE = False
LAST = {}


def _ensure_ntff_hook():
    """The container's antenv lacks axon_hooks; provide a shim module and
    register a ctypes NTFF-profile hook against libaxon_pjrt.so so that
    run_bass_kernel_spmd(trace=True) can capture exec_time_ns."""
    import contextlib
    import ctypes
    import os
    import sys
    import types

    import concourse.bass_utils as bu

    # artifact upload needs a monorepo bucket; stub it out
    bu.upload_artifacts = lambda tmpdir: f"local:{tmpdir}"

    try:
        import antenv.axon_hooks  # noqa: F401

        return
    except ImportError:
        pass

    mod = types.ModuleType("antenv.axon_hooks")
    _h = [None]
    mod.set_axon_ntff_profile_hook = lambda h: _h.__setitem__(0, h)
    mod.get_axon_ntff_profile_hook = lambda: _h[0]
    sys.modules["antenv.axon_hooks"] = mod

    so_path = "/opt/axon/libaxon_pjrt.so"
    if not os.path.exists(so_path):
        return
    lib = ctypes.CDLL(so_path)
    if not hasattr(lib, "axon_start_nrt_profile"):
        return
    lib.axon_start_nrt_profile.argtypes = [
        ctypes.POINTER(ctypes.c_int64),
        ctypes.c_size_t,
    ]
    lib.axon_start_nrt_profile.restype = ctypes.c_int64
    lib.axon_stop_nrt_profile.argtypes = [ctypes.c_char_p]
    lib.axon_stop_nrt_profile.restype = ctypes.c_int64

    @contextlib.contextmanager
    def _hook(output_dir, device_ids):
        import jax

        jax.devices()
        if device_ids:
            ids = (ctypes.c_int64 * len(device_ids))(*device_ids)
            rc = lib.axon_start_nrt_profile(ids, len(device_ids))
        else:
            rc = lib.axon_start_nrt_profile(None, 0)
        if rc != 0:
            raise RuntimeError(f"axon_start_nrt_profile rc={rc}")
        try:
            yield
        finally:
            n = lib.axon_stop_nrt_profile(str(output_dir).encode())
            print(f"ntff profile: {n} file(s) -> {output_dir}", file=sys.stderr)

    mod.set_axon_ntff_profile_hook(_hook)

_wfix_ctr = [0]


def _fix_multi_waits(nc) -> int:
    """This container's walrus rejects >1 sync-wait per instruction: hoist
    extra waits onto single-wait NOPs inserted before the instruction on the
    same engine (earlier same-engine waits preserve semantics exactly)."""
    nsplit = 0
    for fn in nc.m.functions:
        for bb in fn.blocks:
            out = []
            changed = False
            for inst in bb.instructions:
                si = inst.sync_info
                waits = list(si.on_wait) if si is not None else []
                if len(waits) > 1:
                    for w in waits[:-1]:
                        _wfix_ctr[0] += 1
                        nop = mybir.InstNoOp(
                            name=f"I-wfix-{_wfix_ctr[0]}", ins=[], outs=[]
                        )
                        nop.engine = inst.engine
                        nop.sync_info = mybir.SyncInfo(on_wait=[w], on_update=[])
                        out.append(nop)
                    si.on_wait = waits[-1:]
                    nsplit += 1
                    changed = True
                out.append(inst)
            if changed:
                bb.instructions = out
    return nsplit


def _edge_plan(edge_index, n_nodes, npc, nblk):
    """Sort edges by dst per core, pack into [nblk] dst-blocks each padded to
    M tiles of 128 edges.  Returns (M, idx[core], dstl[core], invdeg[core])."""
    src = np.asarray(edge_index[0]).astype(np.int64)
    dst = np.asarray(edge_index[1]).astype(np.int64)
    deg = np.bincount(dst, minlength=n_nodes).astype(np.float32)
    invdeg_full = 1.0 / np.maximum(deg, 1.0)

    per_core = []
    max_count = 0
    for k in range(NCORES):
        lo = k * npc
        m = (dst >= lo) & (dst < lo + npc)
        s = src[m]
        dl = dst[m] - lo
        order = np.argsort(dl, kind="stable")
        s = s[order].astype(np.int32)
        dl = dl[order].astype(np.int32)
        counts = np.bincount(dl >> 7, minlength=nblk)
        max_count = max(max_count, int(counts.max()))
        per_core.append((s, dl, counts))

    M = max(1, -(-max_count // BLK))
    idx_list, dstl_list, invdeg_list = [], [], []
    for k in range(NCORES):
        s, dl, counts = per_core[k]
        src_pad = np.zeros((nblk, M * BLK), np.int32)
        dl_pad = np.full((nblk, M * BLK), 255, np.int32)
        off = np.concatenate([[0], np.cumsum(counts)])
        for b in range(nblk):
            c = counts[b]
            src_pad[b, :c] = s[off[b] : off[b] + c]
            dl_pad[b, :c] = dl[off[b] : off[b] + c] - b * BLK
        # [nblk, M*128] -> [nblk*128 (partition-major rows), M]
        idx_list.append(
            np.ascontiguousarray(
                src_pad.reshape(nblk, M, BLK).transpose(0, 2, 1).reshape(nblk * BLK, M)
            )
        )
        dstl_list.append(
            src_pad.dtype.type(0)  # placeholder, replaced below
        )
        dstl_list[-1] = (
            dl_pad.reshape(nblk, M, BLK)
            .transpose(0, 2, 1)
            .reshape(nblk * BLK, M)
            .astype(np.float32)
            .astype(ml_dtypes.bfloat16)
        )
        v = np.ones(nblk * BLK, np.float32)
        v[:npc] = invdeg_full[k * npc : k * npc + npc]
        invdeg_list.append(np.ascontiguousarray(v.reshape(nblk, BLK).T))
    return M, idx_list, dstl_list, invdeg_list


def _hilo(a32):
    hi = a32.astype(ml_dtypes.bfloat16)
    lo = (a32 - hi.astype(np.float32)).astype(ml_dtypes.bfloat16)
    return np.concatenate([hi, lo], axis=1)


def _agg_block(nc, pools, b, M, t_table, t_idx, t_dstl, iota_sb, invdeg_sb):
    """Emit gather + segment-sum for dst-block b.  Returns mean [128n, d] f32
    (d = table_row/2 per hi/lo half) as an SBUF tile."""
    blkp, selp, msgp, aggps = pools
    width = t_table.shape[1]  # 256 (hi|lo bf16)

    idxb = blkp.tile([BLK, M], I32, tag="idxb")
    nc.sync.dma_start(out=idxb[:], in_=t_idx[b * BLK : (b + 1) * BLK, :])
    dstlb = blkp.tile([BLK, M], BF16, tag="dstlb")
    nc.sync.dma_start(out=dstlb[:], in_=t_dstl[b * BLK : (b + 1) * BLK, :])

    sel = selp.tile([BLK, M * BLK], BF16, tag="sel")
    dstl_ap = dstlb[:]
    dstl_rep = bass.AP(
        tensor=dstl_ap.tensor, offset=dstl_ap.offset, ap=list(dstl_ap.ap) + [[0, BLK]]
    )
    iota_ap = iota_sb[:]
    iota_rep = bass.AP(
        tensor=iota_ap.tensor,
        offset=iota_ap.offset,
        ap=[iota_ap.ap[0], [0, M], iota_ap.ap[1]],
    )
    nc.vector.tensor_tensor(
        out=sel[:].rearrange("p (m j) -> p m j", m=M),
        in0=dstl_rep,
        in1=iota_rep,
        op=mybir.AluOpType.is_equal,
    )

    psum = aggps.tile([BLK, width], F32, tag="agg")
    for t in range(M):
        msg = msgp.tile([BLK, width], BF16, tag="msg")
        nc.gpsimd.indirect_dma_start(
            out=msg[:],
            out_offset=None,
            in_=t_table[:],
            in_offset=bass.IndirectOffsetOnAxis(ap=idxb[:, t : t + 1], axis=0),
        )
        nc.tensor.matmul(
            out=psum[:],
            lhsT=sel[:, t * BLK : (t + 1) * BLK],
            rhs=msg[:],
            start=(t == 0),
            stop=(t == M - 1),
        )

    half = width // 2
    hi = blkp.tile([BLK, half], F32, tag="hi")
    nc.vector.tensor_copy(out=hi[:], in_=psum[:, 0:half])
    sums = blkp.tile([BLK, half], F32, tag="sums")
    nc.vector.tensor_tensor(
        out=sums[:], in0=psum[:, half:width], in1=hi[:], op=mybir.AluOpType.add
    )
    mean = blkp.tile([BLK, half], F32, tag="mean")
    nc.vector.tensor_scalar(
        out=mean[:],
        in0=sums[:],
        scalar1=invdeg_sb[:, b : b + 1],
        scalar2=None,
        op0=mybir.AluOpType.mult,
    )
    return mean


def _build_layer1(n_nodes, nblk, M):
    npad = nblk * BLK
    nc = bass.Bass("TRN2", target_bir_lowering=False, debug=False, num_devices=NCORES)
    t_xh = nc.dram_tensor("xhilo", [n_nodes, 256], BF16, kind="ExternalInput")
    t_xT = nc.dram_tensor("xT", [128, npad], F32, kind="ExternalInput")
    t_idx = nc.dram_tensor("idx", [npad, M], I32, kind="ExternalInput")
    t_dstl = nc.dram_tensor("dstl", [npad, M], BF16, kind="ExternalInput")
    t_invdeg = nc.dram_tensor("invdeg", [128, nblk], F32, kind="ExternalInput")
    t_W1l = nc.dram_tensor("W1l", [128, 128], F32, kind="ExternalInput")
    t_W1r = nc.dram_tensor("W1r", [128, 128], F32, kind="ExternalInput")
    t_b1 = nc.dram_tensor("b1", [128, 1], F32, kind="ExternalInput")
    t_iota = nc.dram_tensor("iota", [128, 128], BF16, kind="ExternalInput")
    t_idf = nc.dram_tensor("identf", [128, 128], F32, kind="ExternalInput")
    t_idb = nc.dram_tensor("identb", [128, 128], BF16, kind="ExternalInput")
    t_hh = nc.dram_tensor("hhilo", [npad, 256], BF16, kind="ExternalOutput")
    t_hfm = nc.dram_tensor("hfm", [128, nblk * 256], BF16, kind="ExternalOutput")

    with tile.TileContext(nc) as tc:
        with (
            tc.tile_pool(name="const", bufs=1) as constp,
            tc.tile_pool(name="blk", bufs=3) as blkp,
            tc.tile_pool(name="selp", bufs=2) as selp,
            tc.tile_pool(name="msgp", bufs=6) as msgp,
            tc.tile_pool(name="aggps", bufs=2, space="PSUM") as aggps,
            tc.tile_pool(name="auxps", bufs=2, space="PSUM") as auxps,
            tc.tile_pool(name="hilops", bufs=1, space="PSUM") as hilops,
            tc.tile_pool(name="hps", bufs=2, space="PSUM") as hps,
        ):
            invdeg_sb = constp.tile([128, nblk], F32)
            nc.sync.dma_start(out=invdeg_sb[:], in_=t_invdeg[:])
            W1l_sb = constp.tile([128, 128], F32)
            nc.sync.dma_start(out=W1l_sb[:], in_=t_W1l[:])
            W1r_sb = constp.tile([128, 128], F32)
            nc.sync.dma_start(out=W1r_sb[:], in_=t_W1r[:])
            b1_sb = constp.tile([128, 1], F32)
            nc.sync.dma_start(out=b1_sb[:], in_=t_b1[:])
            iota_sb = constp.tile([128, 128], BF16)
            nc.sync.dma_start(out=iota_sb[:], in_=t_iota[:])
            idf_sb = constp.tile([128, 128], F32)
            nc.sync.dma_start(out=idf_sb[:], in_=t_idf[:])
            idb_sb = constp.tile([128, 128], BF16)
            nc.sync.dma_start(out=idb_sb[:], in_=t_idb[:])

            pools = (blkp, selp, msgp, aggps)
            for b in range(nblk):
                mean = _agg_block(
                    nc, pools, b, M, t_xh, t_idx, t_dstl, iota_sb, invdeg_sb
                )
                meanT_ps = auxps.tile([128, 128], F32, tag="meanT")
                nc.tensor.transpose(out=meanT_ps[:], in_=mean[:], identity=idf_sb[:])
                meanT = blkp.tile([128, 128], F32, tag="meanT_sb")
                nc.scalar.activation(
                    out=meanT[:], in_=meanT_ps[:], func=mybir.ActivationFunctionType.Copy
                )
                xTb = blkp.tile([128, 128], F32, tag="xTb")
                nc.sync.dma_start(out=xTb[:], in_=t_xT[:, b * BLK : (b + 1) * BLK])
                h_ps = hps.tile([128, 128], F32, tag="hps")
                nc.tensor.matmul(
                    out=h_ps[:], lhsT=W1l_sb[:], rhs=meanT[:], start=True, stop=False
                )
                nc.tensor.matmul(
                    out=h_ps[:], lhsT=W1r_sb[:], rhs=xTb[:], start=False, stop=True
                )
                hT = blkp.tile([128, 128], F32, tag="hT")
                nc.scalar.activation(
                    out=hT[:],
                    in_=h_ps[:],
                    func=mybir.ActivationFunctionType.Relu,
                    bias=b1_sb[:, 0:1],
                )
                # feat-major [hi | lo] staging -> hfm
                fm = blkp.tile([128, 256], BF16, tag="fm")
                nc.vector.tensor_copy(out=fm[:, 0:128], in_=hT[:])
                hif = blkp.tile([128, 128], F32, tag="hif")
                nc.vector.tensor_copy(out=hif[:], in_=fm[:, 0:128])
                nc.vector.tensor_tensor(
                    out=fm[:, 128:256], in0=hT[:], in1=hif[:], op=mybir.AluOpType.subtract
                )
                nc.sync.dma_start(out=t_hfm[:, b * 256 : (b + 1) * 256], in_=fm[:])
                # node-major [hi | lo] rows -> hhilo
                hiT_ps = hilops.tile([128, 128], BF16, tag="hiT")
                nc.tensor.transpose(out=hiT_ps[:], in_=fm[:, 0:128], identity=idb_sb[:])
                loT_ps = hilops.tile([128, 128], BF16, tag="loT")
                nc.tensor.transpose(out=loT_ps[:], in_=fm[:, 128:256], identity=idb_sb[:])
                nm = blkp.tile([128, 256], BF16, tag="nm")
                nc.scalar.activation(
                    out=nm[:, 0:128], in_=hiT_ps[:], func=mybir.ActivationFunctionType.Copy
                )
                nc.scalar.activation(
                    out=nm[:, 128:256], in_=loT_ps[:], func=mybir.ActivationFunctionType.Copy
                )
                nc.sync.dma_start(out=t_hh[b * BLK : (b + 1) * BLK, :], in_=nm[:])
    _fix_multi_waits(nc)
    return nc


def _build_layer2(n_nodes, nblk, M, d_out):
    npad = nblk * BLK
    nc = bass.Bass("TRN2", target_bir_lowering=False, debug=False, num_devices=NCORES)
    t_hh = nc.dram_tensor("hhfull", [n_nodes, 256], BF16, kind="ExternalInput")
    t_hfm = nc.dram_tensor("hfm", [128, nblk * 256], BF16, kind="ExternalInput")
    t_idx = nc.dram_tensor("idx", [npad, M], I32, kind="ExternalInput")
    t_dstl = nc.dram_tensor("dstl", [npad, M], BF16, kind="ExternalInput")
    t_invdeg = nc.dram_tensor("invdeg", [128, nblk], F32, kind="ExternalInput")
    t_W2l = nc.dram_tensor("W2l", [128, d_out], F32, kind="ExternalInput")
    t_W2r = nc.dram_tensor("W2r", [128, d_out], F32, kind="ExternalInput")
    t_b2 = nc.dram_tensor("b2", [d_out, 1], F32, kind="ExternalInput")
    t_iota = nc.dram_tensor("iota", [128, 128], BF16, kind="ExternalInput")
    t_idf = nc.dram_tensor("identf", [128, 128], F32, kind="ExternalInput")
    t_outT = nc.dram_tensor("outT", [d_out, npad], F32, kind="ExternalOutput")

    with tile.TileContext(nc) as tc:
        with (
            tc.tile_pool(name="const", bufs=1) as constp,
            tc.tile_pool(name="blk", bufs=3) as blkp,
            tc.tile_pool(name="selp", bufs=2) as selp,
            tc.tile_pool(name="msgp", bufs=6) as msgp,
            tc.tile_pool(name="aggps", bufs=2, space="PSUM") as aggps,
            tc.tile_pool(name="auxps", bufs=2, space="PSUM") as auxps,
            tc.tile_pool(name="ops", bufs=2, space="PSUM") as ops,
        ):
            invdeg_sb = constp.tile([128, nblk], F32)
            nc.sync.dma_start(out=invdeg_sb[:], in_=t_invdeg[:])
            W2l_sb = constp.tile([128, d_out], F32)
            nc.sync.dma_start(out=W2l_sb[:], in_=t_W2l[:])
            W2r_sb = constp.tile([128, d_out], F32)
            nc.sync.dma_start(out=W2r_sb[:], in_=t_W2r[:])
            b2_sb = constp.tile([d_out, 1], F32)
            nc.sync.dma_start(out=b2_sb[:], in_=t_b2[:])
            iota_sb = constp.tile([128, 128], BF16)
            nc.sync.dma_start(out=iota_sb[:], in_=t_iota[:])
            idf_sb = constp.tile([128, 128], F32)
            nc.sync.dma_start(out=idf_sb[:], in_=t_idf[:])

            pools = (blkp, selp, msgp, aggps)
            for b in range(nblk):
                mean2 = _agg_block(
                    nc, pools, b, M, t_hh, t_idx, t_dstl, iota_sb, invdeg_sb
                )
                m2T_ps = auxps.tile([128, 128], F32, tag="m2T")
                nc.tensor.transpose(out=m2T_ps[:], in_=mean2[:], identity=idf_sb[:])
                m2T = blkp.tile([128, 128], F32, tag="m2T_sb")
                nc.scalar.activation(
                    out=m2T[:], in_=m2T_ps[:], func=mybir.ActivationFunctionType.Copy
                )
                hfmb = blkp.tile([128, 256], BF16, tag="hfmb")
                nc.sync.dma_start(out=hfmb[:], in_=t_hfm[:, b * 256 : (b + 1) * 256])
                hT32 = blkp.tile([128, 128], F32, tag="hT32")
                nc.vector.tensor_tensor(
                    out=hT32[:], in0=hfmb[:, 0:128], in1=hfmb[:, 128:256],
                    op=mybir.AluOpType.add,
                )
                o_ps = ops.tile([d_out, 128], F32, tag="ops")
                nc.tensor.matmul(
                    out=o_ps[:], lhsT=W2l_sb[:], rhs=m2T[:], start=True, stop=False
                )
                nc.tensor.matmul(
                    out=o_ps[:], lhsT=W2r_sb[:], rhs=hT32[:], start=False, stop=True
                )
                outb = blkp.tile([d_out, 128], F32, tag="outb")
                nc.vector.tensor_scalar(
                    out=outb[:],
                    in0=o_ps[:],
                    scalar1=b2_sb[:, 0:1],
                    scalar2=None,
                    op0=mybir.AluOpType.add,
                )
                nc.sync.dma_start(out=t_outT[:, b * BLK : (b + 1) * BLK], in_=outb[:])
    _fix_multi_waits(nc)
    return nc


def kernel(x, W1_l, b1, W1_r, W2_l, b2, W2_r, edge_index):
    x = np.asarray(x, dtype=np.float32)
    W1_l = np.asarray(W1_l, dtype=np.float32)
    b1 = np.asarray(b1, dtype=np.float32)
    W1_r = np.asarray(W1_r, dtype=np.float32)
    W2_l = np.asarray(W2_l, dtype=np.float32)
    b2 = np.asarray(b2, dtype=np.float32)
    W2_r = np.asarray(W2_r, dtype=np.float32)

    n_nodes, d_in = x.shape
    d_hid = W1_l.shape[1]
    d_out = W2_l.shape[1]
    assert d_in == 128 and d_hid == 128, "kernel specialized for d_in=d_hid=128"
    npc = n_nodes // NCORES
    nblk = -(-npc // BLK)
    npad = nblk * BLK

    M, idx_l, dstl_l, invdeg_l = _edge_plan(edge_index, n_nodes, npc, nblk)

    xhilo = _hilo(x)
    xT_l = []
    for k in range(NCORES):
        a = np.zeros((d_in, npad), np.float32)
        a[:, :npc] = x[k * npc : (k + 1) * npc].T
        xT_l.append(a)

    iota = np.broadcast_to(
        np.arange(128, dtype=np.float32), (128, 128)
    ).astype(ml_dtypes.bfloat16)
    identf = np.eye(128, dtype=np.float32)
    identb = np.eye(128, dtype=np.float32).astype(ml_dtypes.bfloat16)

    if PROFILE:
        _ensure_ntff_hook()

    nc1 = _build_layer1(n_nodes, nblk, M)
    in_maps1 = [
        {
            "xhilo": xhilo,
            "xT": xT_l[k],
            "idx": idx_l[k],
            "dstl": dstl_l[k],
            "invdeg": invdeg_l[k],
            "W1l": W1_l,
            "W1r": W1_r,
            "b1": b1.reshape(d_hid, 1),
            "iota": iota,
            "identf": identf,
            "identb": identb,
        }
        for k in range(NCORES)
    ]
    res1 = run_bass_kernel_spmd(
        nc1, in_maps1, list(range(NCORES)), trace=PROFILE
    )
    if PROFILE:
        LAST["l1_ns"] = res1.exec_time_ns

    hh_full = np.concatenate(
        [res1.results[k]["hhilo"][:npc] for k in range(NCORES)], axis=0
    )
    hfm_l = [res1.results[k]["hfm"] for k in range(NCORES)]

    nc2 = _build_layer2(n_nodes, nblk, M, d_out)
    in_maps2 = [
        {
            "hhfull": hh_full,
            "hfm": hfm_l[k],
            "idx": idx_l[k],
            "dstl": dstl_l[k],
            "invdeg": invdeg_l[k],
            "W2l": W2_l,
            "W2r": W2_r,
            "b2": b2.reshape(d_out, 1),
            "iota": iota,
            "identf": identf,
        }
        for k in range(NCORES)
    ]
    res2 = run_bass_kernel_spmd(
        nc2, in_maps2, list(range(NCORES)), trace=PROFILE
    )
    if PROFILE:
        LAST["l2_ns"] = res2.exec_time_ns

    out = np.empty((n_nodes, d_out), np.float32)
    for k in range(NCORES):
        out[k * npc : (k + 1) * npc] = res2.results[k]["outT"][:, :npc].T
    return out
